# revision 54
# baseline (speedup 1.0000x reference)
"""Multi-head attention (RoPE, causal) Trainium2 kernel — v2.

Problem: B=2, L=2048, D=2048, H=16, dh=128, fp32.
Sharding: 8 cores = 2 batches x 4 head-groups (4 heads/core).
Each core computes QKV projections for its heads, RoPE, causal
attention, and a partial output projection (its heads' rows of Wo);
the host sums the 4 partials per batch.

v2 changes vs baseline:
 - all matmul operands fp16 (1 cyc/row like bf16, better mantissa,
   half the DMA, DVE 4x fast mode for fp16 SBUF-only elementwise ops)
 - weights SBUF-resident, DMA'd once (baseline re-loaded every chunk)
 - V projection emits [l, 4*dh] tiles via 16-step chains at ap=512
   (256 matmuls instead of 512 at ap=256)
 - softmax denominator: DVE-accumulated esum (fp16) + ONE ones-matmul
   per (head, chunk) instead of a PE matmul per k-tile (saves ~45us PE)
 - emission order software-pipelines PE work so the tensor engine
   never idles (idle gaps drop it to the 1.2GHz mid p-state)
 - rotate-half via two Act-engine half-copies (no PE matmul, no extra
   PSUM bank); RoPE combine on DVE
 - Wo grouped for stationary reuse (2-bank otp pairs) and interleaved
   early into the next chunk's attention phase; out written fp16
   (host upcasts + sums partials)
 - DMA: x pre-tiled on host (contiguous rows), chunk-0 split into
   eighths so chains trickle-start; transfers spread across the three
   issue rings (sync/gpsimd/scalar); deferred loads (wk/wv/wo, x
   prefetch) are gated behind dummy-dep writes because the rings
   round-robin all queued transfers
"""
import sys
import numpy as np

sys.path.insert(0, '/opt/trn_rl_repo')

import concourse.bass as bass  # noqa: E402,F401
import concourse.mybir as mybir  # noqa: E402
import concourse.tile as tile  # noqa: E402
from concourse import bacc  # noqa: E402
from concourse import library_config  # noqa: E402
from concourse.bass_utils import run_bass_kernel_spmd  # noqa: E402

B, L, D = 2, 2048, 2048
H, DH = 16, 128
HG = 4           # heads per core
G = H // HG      # head groups (cores per batch)
NCORES = 8
CHUNK = 512      # l-chunk
NCH = L // CHUNK          # 4 chunks
KT = D // 128             # 16 k-tiles over D
LT = L // 128             # 16 l-tiles
ROPE_BASE = 10000.0

f32 = mybir.dt.float32
f16 = mybir.dt.float16

_built = None
PHASES = []


def _build():
    nc = bacc.Bacc()

    # x pre-tiled on host: [c][p][kt][n] = x^T[kt*128 + p, c*512 + n]
    # (16KB contiguous per partition row; chunk-0 sliced into eighths)
    xt_d = nc.declare_dram_parameter("xt", [NCH, 128, KT, CHUNK], f16,
                                     isOutput=False)
    # wq/wk: [p][m*2048 + kt*128 + f] = W^T[kt*128+p, m*128+f] (deinterleaved
    # rows, scale folded into wq)
    wq_d = nc.declare_dram_parameter("wq", [128, HG * D], f16, isOutput=False)
    wk_d = nc.declare_dram_parameter("wk", [128, HG * D], f16, isOutput=False)
    # wv moving layout: [p][kt*512 + f] = Wv^T[kt*128+p, hs0*128 + f]
    wv_d = nc.declare_dram_parameter("wv", [128, KT * 512], f16, isOutput=False)
    # wo: [p][h*2048 + o] = Wo[o, (hs0+h)*128 + p]
    wo_d = nc.declare_dram_parameter("wo", [128, HG * D], f16, isOutput=False)
    cos_d = nc.declare_dram_parameter("cosT", [128, L], f16, isOutput=False)
    sin_d = nc.declare_dram_parameter("sinT", [128, L], f16, isOutput=False)
    tri_d = nc.declare_dram_parameter("tri", [128, 128], f16, isOutput=False)
    ones_c_d = nc.declare_dram_parameter("ones_c", [128, 1], f16, isOutput=False)

    out_d = nc.declare_dram_parameter("out", [L, D], f16, isOutput=True)

    with tile.TileContext(nc) as tc:
        with (
            tc.tile_pool(name="const", bufs=1) as const,
            tc.tile_pool(name="persist", bufs=1) as persist,
            tc.tile_pool(name="xs", bufs=9) as xs,             # chunk-0 eighths
            tc.tile_pool(name="xsb", bufs=2) as xsb,           # full x chunks
            tc.tile_pool(name="qt", bufs=8) as qtp,            # qt per chunk
            tc.tile_pool(name="at", bufs=8) as atp,            # at per chunk
            tc.tile_pool(name="qraw", bufs=3) as qrawp,        # PSUM->SBUF f16
            tc.tile_pool(name="t12", bufs=4) as t12p,          # rope transients
            tc.tile_pool(name="et", bufs=8) as etp,            # exp tiles
            tc.tile_pool(name="es", bufs=3) as esp,            # esum tiles
            tc.tile_pool(name="sm", bufs=3) as smp,            # recip [1,512]
            tc.tile_pool(name="bc", bufs=2) as bcp,            # bcast [128,512]
            tc.tile_pool(name="ob", bufs=8) as obp,            # out staging
            tc.tile_pool(name="pacc", bufs=3, space="PSUM") as pacc,
            tc.tile_pool(name="pst", bufs=2, space="PSUM") as pst,
            tc.tile_pool(name="put", bufs=2, space="PSUM") as put,
            tc.tile_pool(name="prb", bufs=1, space="PSUM") as prb,
        ):
            # ---- constants / weights (one-time DMA, spread across issue
            # queues so chunk-0 x loads aren't stuck behind them) ----
            nc.gpsimd.load_library(library_config.attn)
            wq_t = const.tile([128, HG * D], f16)
            wk_t = const.tile([128, HG * D], f16)
            wv_t = const.tile([128, KT * 512], f16)
            wo_t = const.tile([128, HG * D], f16)
            cos_t = const.tile([128, L], f16)
            sin_t = const.tile([128, L], f16)
            tri_t = const.tile([128, 128], f16)
            ones_c = const.tile([128, 1], f16)
            # critical path to first matmuls: wq strips (scalar ring),
            # x eighths (sync+gpsimd rings, emitted in the c-loop). wk/wv/wo
            # are issued later from the Act queue (naturally serialized with
            # compute) so they don't eat HBM bandwidth during startup.
            for m in range(HG):
                nc.scalar.dma_start(out=wq_t[:, m * D:(m + 1) * D],
                                    in_=wq_d[:, m * D:(m + 1) * D])
            nc.scalar.dma_start(out=cos_t[:], in_=cos_d[:])
            nc.scalar.dma_start(out=sin_t[:], in_=sin_d[:])
            nc.gpsimd.dma_start(out=ones_c[:], in_=ones_c_d[:])
            nc.gpsimd.dma_start(out=tri_t[:], in_=tri_d[:])

            # ---- persistent activations (full history) ----
            kt_t = [persist.tile([128, L], f16, name=f"ktt{h}") for h in range(HG)]
            v_t = [persist.tile([128, HG * 128], f16, name=f"vt{lt}")
                   for lt in range(LT)]

            def w_ap(wt, m, kt):
                return wt[:, m * D + kt * 128:m * D + kt * 128 + 128]

            at_tiles = {}      # (c, h) -> at tile
            qt_tiles = {}      # (c, h) -> qt tile

            def emit_wo_group(cc, sl):
                """Output projection for l-tile (cc*4+sl): 16 matmuls in two
                2-bank passes; at-slice stationary reused over 2 ot movings."""
                mt = cc * 4 + sl
                for otp in range(2):
                    wops = [pacc.tile([128, 512], f32, tag="acc",
                                      name=f"wop{mt}_{otp}_{oi}")
                            for oi in range(2)]
                    for h in range(HG):
                        a_sl = at_tiles[(cc, h)][:, sl * 128:(sl + 1) * 128]
                        for oi in range(2):
                            ot = otp * 2 + oi
                            nc.tensor.matmul(
                                wops[oi][:], a_sl,
                                wo_t[:, h * D + ot * 512:h * D + (ot + 1) * 512],
                                start=(h == 0), stop=(h == HG - 1))
                    for oi in range(2):
                        ot = otp * 2 + oi
                        osb = obp.tile([128, 512], f16, tag="ob")
                        nc.scalar.copy(out=osb[:], in_=wops[oi][:])
                        if cc == NCH - 1:
                            # final chunk: finer pieces across both rings so
                            # the trailing drain ends sooner
                            for hf in range(2):
                                q_eng = nc.sync if (2 * ot + hf) % 2 == 0 \
                                    else nc.gpsimd
                                q_eng.dma_start(
                                    out=out_d[mt * 128:(mt + 1) * 128,
                                              ot * 512 + hf * 256:
                                              ot * 512 + (hf + 1) * 256],
                                    in_=osb[:, hf * 256:(hf + 1) * 256])
                        else:
                            q_eng = nc.sync if (ot % 2 == 0) else nc.gpsimd
                            q_eng.dma_start(
                                out=out_d[mt * 128:(mt + 1) * 128,
                                          ot * 512:(ot + 1) * 512],
                                in_=osb[:])

            for c in range(NCH):
                PHASES.append((f"c{c}_load", int(nc.next_id())))
                cs = slice(c * CHUNK, (c + 1) * CHUNK)
                if c == 0:
                    # eighth-granular so chains trickle-start as x arrives
                    xc = []
                    for hh in range(8):
                        xq = xs.tile([128, 2, CHUNK], f16, tag="xc")
                        q_eng = nc.sync if hh % 2 == 0 else nc.gpsimd
                        q_eng.dma_start(out=xq[:],
                                        in_=xt_d[c][:, 2 * hh:2 * hh + 2, :])
                        xc.append(xq)

                    def gate(dst_ap, anchor_ap):
                        # dummy write: defers the next DMA on dst until the
                        # anchor data has landed (rings round-robin all
                        # queued transfers, so un-gated issues steal startup
                        # bandwidth)
                        nc.vector.tensor_copy(out=dst_ap, in_=anchor_ap)

                    x_anchor = xc[5][0:1, 0:1, 0:1].rearrange("p a b -> p (a b)")
                    x_anchor7 = xc[7][0:1, 0:1, 0:1].rearrange("p a b -> p (a b)")
                    for m in range(HG):
                        gate(wk_t[0:1, m * D:m * D + 1],
                             x_anchor if m < 2 else x_anchor7)
                        nc.gpsimd.dma_start(out=wk_t[:, m * D:(m + 1) * D],
                                            in_=wk_d[:, m * D:(m + 1) * D])
                    gate(wv_t[0:1, 0:1], x_anchor7)
                    nc.gpsimd.dma_start(out=wv_t[:], in_=wv_d[:])

                    def xtile(kt, xc=xc):
                        return xc[kt // 2][:, kt % 2, :]
                else:
                    xbig = xsb.tile([128, KT, CHUNK], f16, tag="xbig")
                    anchor = qt_tiles[(c - 1, 0)][0:1, 0:1]
                    nc.vector.tensor_copy(
                        out=xbig[0:1, 0:1, 0:1].rearrange("p a b -> p (a b)"),
                        in_=anchor)
                    q_eng = nc.sync if c % 2 == 1 else nc.gpsimd
                    q_eng.dma_start(out=xbig[:], in_=xt_d[c])

                    def xtile(kt, xbig=xbig):
                        return xbig[:, kt, :]

                PHASES.append((f"c{c}_qk", int(nc.next_id())))
                # ---------- Q/K projections; RoPE pipelined one stage behind --
                pending = None   # (ps, dst_ap)

                def rope_copy(ps):
                    # rotate-half via two Act half-copies (R @ raw):
                    # qrot[0:64] = -raw[64:128]; qrot[64:128] = raw[0:64]
                    qrot = qrawp.tile([128, CHUNK], f16, tag="qraw")
                    nc.scalar.activation(qrot[0:64, :], ps[64:128, :],
                                         mybir.ActivationFunctionType.Copy,
                                         scale=-1.0)
                    nc.scalar.activation(qrot[64:128, :], ps[0:64, :],
                                         mybir.ActivationFunctionType.Copy)
                    return qrot

                def rope_rest(ps, qrot, dst_ap):
                    # dst = raw*cos + rot(raw)*sin
                    t1 = t12p.tile([128, CHUNK], f16, tag="t12")
                    nc.vector.tensor_tensor(out=t1[:], in0=ps[:],
                                            in1=cos_t[:, cs],
                                            op=mybir.AluOpType.mult)
                    t2 = t12p.tile([128, CHUNK], f16, tag="t12")
                    nc.vector.tensor_tensor(out=t2[:], in0=qrot[:],
                                            in1=sin_t[:, cs],
                                            op=mybir.AluOpType.mult)
                    nc.vector.tensor_tensor(out=dst_ap, in0=t1[:], in1=t2[:],
                                            op=mybir.AluOpType.add)

                def emit_rope(ps, dst_ap):
                    rope_rest(ps, rope_copy(ps), dst_ap)

                def qk_dst(i):
                    m = i % HG
                    if i < HG:
                        qt = qtp.tile([128, CHUNK], f16, tag="qt",
                                      name=f"qt_c{c}_h{m}")
                        qt_tiles[(c, m)] = qt
                        return qt[:]
                    return kt_t[m][:, cs]

                if c == 0:
                    # startup: chains split in kt halves so the first halves
                    # run on the first 4 x-eighths while the rest stream in
                    jobs = [(wq_t, m) for m in range(HG)] + \
                           [(wk_t, m) for m in range(HG)]
                    ps_of = {}

                    def half_chain(i, lo, hi):
                        wt, m = jobs[i]
                        if lo == 0:
                            ps_of[i] = pacc.tile([128, CHUNK], f32, tag="acc",
                                                 name=f"ps0_{i}")
                        ps = ps_of[i]
                        for kt in range(lo, hi):
                            nc.tensor.matmul(ps[:], w_ap(wt, m, kt), xtile(kt),
                                             start=(kt == 0),
                                             stop=(kt == KT - 1))

                    for i in range(3):
                        half_chain(i, 0, 8)
                    for i in range(8):
                        half_chain(i, 8, 16)
                        emit_rope(ps_of[i], qk_dst(i))
                        if i + 3 < 8:
                            half_chain(i + 3, 0, 8)
                else:
                    for i in range(2 * HG):
                        wt = wq_t if i < HG else wk_t
                        m = i % HG
                        ps = pacc.tile([128, CHUNK], f32, tag="acc")
                        for kt in range(KT):
                            nc.tensor.matmul(ps[:], w_ap(wt, m, kt), xtile(kt),
                                             start=(kt == 0), stop=(kt == KT - 1))
                        if pending is not None:
                            emit_rope(*pending)
                        pending = (ps, qk_dst(i))

                PHASES.append((f"c{c}_v", int(nc.next_id())))
                # ---------- V projection: v_t[lt] = [l(128), 4 heads * 128] ----
                if c == 0:
                    # deferred wo load: needed first at the attn(1) interleave
                    nc.scalar.dma_start(out=wo_t[:], in_=wo_d[:])
                for sl in range(CHUNK // 128):
                    lt = c * (CHUNK // 128) + sl
                    vps = pacc.tile([128, 512], f32, tag="acc")
                    for kt in range(KT):
                        nc.tensor.matmul(
                            vps[:], xtile(kt)[:, sl * 128:(sl + 1) * 128],
                            wv_t[:, kt * 512:(kt + 1) * 512],
                            start=(kt == 0), stop=(kt == KT - 1))
                    if pending is not None:
                        emit_rope(*pending)
                        pending = None
                    nc.scalar.copy(v_t[lt][:], vps[:])

                PHASES.append((f"c{c}_attn", int(nc.next_id())))
                # ---------- attention for q-chunk c ----------
                nkt = (c + 1) * (CHUNK // 128)   # causal: k-tiles 0..nkt-1
                for h in range(HG):
                    qt = qt_tiles[(c, h)]
                    ut = put.tile([128, CHUNK], f32, tag="ut")
                    esum = esp.tile([128, CHUNK], f16, tag="es")
                    for kt in range(nkt):
                        # causal fine-grain: diag tile j only covers q >= 128j
                        diag_j = kt - c * 4
                        qs = (slice(diag_j * 128, CHUNK) if diag_j > 0
                              else slice(0, CHUNK))
                        st = pst.tile([128, CHUNK], f32, tag="st")
                        nc.tensor.matmul(st[:, qs],
                                         kt_t[h][:, kt * 128:(kt + 1) * 128],
                                         qt[:, qs], start=True, stop=True)
                        et = etp.tile([128, CHUNK], f16, tag="et")
                        nc.scalar.activation(et[:, qs], st[:, qs],
                                             mybir.ActivationFunctionType.Exp)
                        if diag_j >= 0:
                            js = slice(diag_j * 128, (diag_j + 1) * 128)
                            nc.vector.tensor_tensor(
                                out=et[:, js], in0=et[:, js], in1=tri_t[:],
                                op=mybir.AluOpType.mult)
                        nc.tensor.matmul(ut[:, qs],
                                         v_t[kt][:, h * 128:(h + 1) * 128],
                                         et[:, qs], start=(kt == 0),
                                         stop=(kt == nkt - 1),
                                         skip_group_check=True)
                        if kt == 0:
                            nc.vector.tensor_copy(out=esum[:], in_=et[:])
                        else:
                            nc.vector.tensor_tensor(out=esum[:, qs],
                                                    in0=esum[:, qs],
                                                    in1=et[:, qs],
                                                    op=mybir.AluOpType.add)
                    rs = prb.tile([1, CHUNK], f32, tag="rb")
                    nc.tensor.matmul(rs[:], ones_c[:], esum[:],
                                     start=True, stop=True)
                    recip = smp.tile([1, CHUNK], f32, tag="recip")
                    nc.vector.reciprocal_approx_fast(out=recip[:], in_=rs[:])
                    bc_sb = bcp.tile([128, CHUNK], f32, tag="bc")
                    nc.gpsimd.partition_broadcast(bc_sb[:], recip[:])
                    at = atp.tile([128, CHUNK], f16, tag="at",
                                  name=f"at_c{c}_h{h}")
                    at_tiles[(c, h)] = at
                    nc.vector.tensor_tensor(out=at[:], in0=ut[:],
                                            in1=bc_sb[:],
                                            op=mybir.AluOpType.mult)
                    # interleave previous chunk's output projection early in
                    # the chunk so its PSUM slots/evictions clear before the
                    # chunk-end at-chain
                    if c >= 1 and h < 2:
                        emit_wo_group(c - 1, 2 * h)
                        emit_wo_group(c - 1, 2 * h + 1)

                if c == NCH - 1:
                    PHASES.append((f"c{c}_wo", int(nc.next_id())))
                    for sl in range(4):
                        emit_wo_group(c, sl)

    nc.finalize()
    return nc


def _get_nc():
    global _built
    if _built is None:
        _built = _build()
    return _built


def _host_prep(x, positions, Wq, Wk, Wv, Wo):
    """Build per-core input maps."""
    x = np.asarray(x, np.float32)
    positions = np.asarray(positions)
    Wq = np.asarray(Wq, np.float32)
    Wk = np.asarray(Wk, np.float32)
    Wv = np.asarray(Wv, np.float32)
    Wo = np.asarray(Wo, np.float32)

    scale = np.float32(1.0 / np.sqrt(DH))
    perm = np.concatenate([np.arange(0, DH, 2), np.arange(1, DH, 2)])  # deinterleave

    Wq_p = (Wq * scale).reshape(H, DH, D)[:, perm, :]   # [H, dh, D]
    Wk_p = Wk.reshape(H, DH, D)[:, perm, :]

    # RoPE tables per batch (deinterleaved: first 64 = even dims, last 64 = odd)
    inv_freq = 1.0 / (ROPE_BASE ** (np.arange(0, DH, 2, dtype=np.float32) / DH))
    cosT = np.empty((B, 128, L), np.float32)
    sinT = np.empty((B, 128, L), np.float32)
    for b in range(B):
        freqs = positions[b].astype(np.float32)[:, None] * inv_freq[None, :]  # [L, 64]
        cb = np.cos(freqs).T.astype(np.float32)  # [64, L]
        sb = np.sin(freqs).T.astype(np.float32)
        cosT[b] = np.concatenate([cb, cb], axis=0)
        sinT[b] = np.concatenate([sb, sb], axis=0)

    # causal block mask (0/1, exact in fp16): tri[k, q] = k <= q
    tri = (np.arange(128)[:, None] <= np.arange(128)[None, :]).astype(np.float16)
    ones_c = np.ones((128, 1), np.float16)

    in_maps = []
    for core in range(NCORES):
        b, g = divmod(core, G)
        hs = slice(g * HG, (g + 1) * HG)
        # W^T for this core's heads: [D, HG*dh]
        wqT = Wq_p[hs].reshape(HG * DH, D).T          # [D, 512]
        wkT = Wk_p[hs].reshape(HG * DH, D).T
        wvT = Wv.reshape(H, DH, D)[hs].reshape(HG * DH, D).T
        # wq/wk: [p][m*2048 + kt*128 + f] = wT[kt*128+p, m*128+f]
        wq_c = np.ascontiguousarray(
            wqT.reshape(KT, 128, HG, DH).transpose(1, 2, 0, 3).reshape(
                128, HG * D)).astype(np.float16)
        wk_c = np.ascontiguousarray(
            wkT.reshape(KT, 128, HG, DH).transpose(1, 2, 0, 3).reshape(
                128, HG * D)).astype(np.float16)
        # wv: [p][kt*512 + f] = wvT[kt*128+p, f]
        wv_c = np.ascontiguousarray(
            wvT.reshape(KT, 128, 512).transpose(1, 0, 2).reshape(
                128, KT * 512)).astype(np.float16)
        # wo: [p][h*2048 + o] = Wo[o, (g*HG+h)*dh + p]
        wo_c = np.ascontiguousarray(
            Wo.T.reshape(H, DH, D)[hs].transpose(1, 0, 2).reshape(
                DH, HG * D)).astype(np.float16)
        # x pre-tiled: [c][p][kt][n] = x^T[kt*128 + p, c*512 + n]
        xtb = x[b].T.astype(np.float16).reshape(KT, 128, NCH, CHUNK)
        xtb = np.ascontiguousarray(xtb.transpose(2, 1, 0, 3))
        in_maps.append({
            "xt": xtb,
            "wq": wq_c, "wk": wk_c, "wv": wv_c, "wo": wo_c,
            "cosT": cosT[b].astype(np.float16),
            "sinT": sinT[b].astype(np.float16),
            "tri": tri, "ones_c": ones_c,
        })
    return in_maps


def kernel(x, positions, Wq, Wk, Wv, Wo, _profile=False):
    nc = _get_nc()
    in_maps = _host_prep(x, positions, Wq, Wk, Wv, Wo)
    res = run_bass_kernel_spmd(nc, in_maps, list(range(NCORES)), trace=_profile)
    out = np.zeros((B, L, D), np.float32)
    for core in range(NCORES):
        b = core // G
        out[b] += res.results[core]["out"].astype(np.float32)
    if _profile:
        kernel._last_exec_time_ns = res.exec_time_ns
        kernel._last_trace = res.instructions_and_trace
    return out
